# revision 30
# baseline (speedup 1.0000x reference)
"""Trainium2 Bass kernel for nn_GAT_87617332838818.  (original baseline)"""

import os
import sys

import numpy as np

for _p in ("/root/.axon_site/_ro/trn_rl_repo", "/opt/trn_rl_repo"):
    if os.path.isdir(_p) and _p not in sys.path:
        sys.path.append(_p)

import concourse.bass as bass
import concourse.tile as tile
from concourse import bacc, mybir
from concourse.bass_utils import run_bass_kernel_spmd

N_CORES = 8
N_PER = 6250            # 50000 / 8
D_IN = 128
D_HID = 96
D_OUT = 40
MM_N = 512              # matmul moving free-dim (1 PSUM bank)
FDP = 512               # group free-dim (1 PSUM bank)

F16 = mybir.dt.float16
BF16 = mybir.dt.bfloat16
F32 = mybir.dt.float32

Act = mybir.ActivationFunctionType
Alu = mybir.AluOpType

_pairs = [FDP] * (N_PER // FDP)
if N_PER % FDP:
    _pairs.append(N_PER % FDP)
P = len(_pairs)
_pstarts = [sum(_pairs[:i]) for i in range(P)]

# ACT is the steady-state pacer (~20.5us vs DVE ~19.6): keep only 5 L0
# relus on ACT (pair 4 and the head-time remainder pair moved to DVE)
R_DRAIN_ON_ACT = tuple((p, 0) for p in range(P)
                       if p % 4 not in (2, 3) and p not in (4, P - 1))
OUT_DRAIN_ON_ACT = (11, 12)
# pairs 0 | 1-4 | 5-8 | 9-11; the remainder pair P-1 gets its own small
# head DMA and is processed entirely inside the warmup window.
X_BATCHES = [1, 4, 4, 3]
N_WARMUP_MM = int(os.environ.get("GAT_WARMUP", "12"))
REM = P - 1

_batch_of = {}
_b0 = 0
for _bi, _bn in enumerate(X_BATCHES):
    for _g in range(_b0, min(_b0 + _bn, P - 1)):
        _batch_of[_g] = _bi
    _b0 += _bn
assert _b0 >= P - 1


def _mm_splits(fd):
    """Split a pair-tick's fd into <=512 matmul chunks."""
    out = []
    j = 0
    while j < fd:
        out.append((j, min(j + MM_N, fd)))
        j += MM_N
    return out


def _build_program() -> bass.Bass:
    nc = bacc.Bacc(None, target_bir_lowering=False, debug=False)

    xw = nc.declare_dram_parameter("xw", [D_IN, D_HID + N_PER], F16,
                                   isOutput=False)
    wb = nc.declare_dram_parameter("wb", [D_HID, D_HID + D_OUT], BF16,
                                   isOutput=False)
    yT = nc.declare_dram_parameter("yT", [104, 3178], F16, isOutput=True)

    st = {}
    st_batch = {}
    batch_tiles = {}

    with tile.TileContext(nc) as tc:
        with (
            tc.tile_pool(name="consts", bufs=1) as consts,
            tc.tile_pool(name="x0", bufs=1) as x0pool,
            tc.tile_pool(name="xin", bufs=2) as xpool,
            tc.tile_pool(name="sb", bufs=3) as sb,
            tc.tile_pool(name="ps0", bufs=3, space="PSUM") as ps0,
            tc.tile_pool(name="ps1", bufs=3, space="PSUM") as ps1,
            tc.tile_pool(name="ps2", bufs=2, space="PSUM") as ps2,
        ):
            # --- PE warm-up on garbage SBUF during the DMA-bound head.
            junk_w = consts.tile([D_IN, D_OUT], F16, tag="junkw")
            junk_x = consts.tile([D_IN, MM_N], F16, tag="junkx")
            nc.gpsimd.memset(junk_w[:], 0.0)
            nc.gpsimd.memset(junk_x[:], 0.0)
            # remainder pair's x rides the gpsimd queue (cheap DMA issue)
            # in parallel with batch-0 on sync, just after the memsets so
            # the PE warmup isn't delayed behind the DMA issue
            xtr = x0pool.tile([D_IN, _pairs[P - 1]], F16, tag="xtr")
            nc.gpsimd.dma_start(xtr[:, :_pairs[P - 1]],
                                xw[:, D_HID + _pstarts[P - 1]:
                                   D_HID + _pstarts[P - 1] + _pairs[P - 1]])
            warm = ps2.tile([104, MM_N], F32, tag="p2")

            def junk_mms(n, moving=None):
                # moving=real-x makes the junk DEPEND on the batch-0 DMA,
                # so the scheduler cannot hoist junk ahead of the first
                # real L0 matmul (which would delay ACT's first drain).
                mv = junk_x[:] if moving is None else moving
                for _ in range(n):
                    nc.tensor.matmul(warm[:D_OUT], junk_w[:], mv,
                                     start=True, stop=True)

            wb_sb = consts.tile([D_HID, D_HID + D_OUT], BF16, tag="wb")
            w1_sb = wb_sb[:, :D_HID]
            w2_sb = wb_sb[:, D_HID:D_HID + D_OUT]

            def relu_drain(out_ap, psum_ap, on_act):
                """out = max(psum, 0), PSUM -> SBUF bf16."""
                if on_act:
                    nc.scalar.activation(out_ap, psum_ap, Act.Relu)
                else:
                    nc.vector.tensor_scalar_max(out_ap, psum_ap, 0.0)

            def exp_elu(p, lyr, psum, fd):
                # the remainder pair gets dedicated tags so its head-time
                # drains never WAR-wait on the main pipeline's slot reuse
                sfx = "z" if p == REM else ""
                e = sb.tile([D_HID, fd], BF16, tag=f"e{lyr}{sfx}")
                r = sb.tile([D_HID, fd], BF16, tag=f"r{lyr}{sfx}")
                for j0, j1 in _mm_splits(fd):
                    nc.scalar.activation(e[:, j0:j1], psum[:, j0:j1], Act.Exp)
                    relu_drain(r[:, j0:j1], psum[:, j0:j1],
                               (p, lyr) in R_DRAIN_ON_ACT)
                t = sb.tile([D_HID, fd], BF16, tag=f"t{lyr}{sfx}")
                nc.vector.tensor_scalar(t[:, :fd], e[:, :fd], 1.0, -1.0,
                                        Alu.min, Alu.add)
                return r, t

            def stage_load(p):
                bi = _batch_of[p]
                if p > 0 and _batch_of[p - 1] == bi:
                    st[p] = st_batch[bi]
                    return
                p1_ = p
                while p1_ + 1 < P - 1 and _batch_of[p1_ + 1] == bi:
                    p1_ += 1
                lo = _pstarts[p] + (0 if bi else -D_HID)   # batch 0 incl. w0
                hi = _pstarts[p1_] + _pairs[p1_]
                cols = hi - lo
                pool = x0pool if bi == 0 else xpool
                width = D_HID + FDP * X_BATCHES[0] if bi == 0 else FDP * 4
                xt = pool.tile([D_IN, width], F16,
                               tag=("xt0" if bi == 0 else "xt"))
                nc.sync.dma_start(xt[:, :cols], xw[:, D_HID + lo:D_HID + hi])
                st_batch[bi] = {"xt": xt, "base": lo}
                st[p] = st_batch[bi]

            def stage0_mm(p):
                fd = _pairs[p]
                s = dict(st[p])
                st[p] = s
                xo = _pstarts[p] - s["base"]
                w0_sb = batch_tiles["w0"]
                p0 = ps0.tile([D_HID, FDP], F32, tag="p0")
                for j0, j1 in _mm_splits(fd):
                    nc.tensor.matmul(p0[:, j0:j1], w0_sb,
                                     s["xt"][:, xo + j0:xo + j1],
                                     start=True, stop=True)
                s["p0"] = p0

            def stage0_elu(p):
                s = st[p]
                s["r1"], s["t1"] = exp_elu(p, 0, s.pop("p0"), _pairs[p])

            def stage1_mm(p):
                fd = _pairs[p]
                s = st[p]
                p1 = ps1.tile([D_HID, FDP], F32, tag="p1")
                for j0, j1 in _mm_splits(fd):
                    nc.tensor.matmul(p1[:, j0:j1], w1_sb, s["r1"][:, j0:j1],
                                     start=True, stop=False)
                    nc.tensor.matmul(p1[:, j0:j1], w1_sb, s["t1"][:, j0:j1],
                                     start=False, stop=True)
                s["p1"] = p1

            def stage1_elu(p):
                s = st[p]
                s["r2"], s["t2"] = exp_elu(p, 1, s.pop("p1"), _pairs[p])

            pair_state = {}

            def stage2(p):
                fd = _pairs[p]
                s = st.pop(p)
                if p % 2 == 0:
                    p2 = ps2.tile([104, FDP], F32, tag="p2")
                    pair_state[p // 2] = p2
                    rows = slice(0, D_OUT)
                else:
                    p2 = pair_state[p // 2]
                    rows = slice(64, 64 + D_OUT)
                nc.tensor.matmul(p2[rows, :fd], w2_sb, s["r2"][:, :fd],
                                 start=True, stop=False)
                nc.tensor.matmul(p2[rows, :fd], w2_sb, s["t2"][:, :fd],
                                 start=False, stop=True)
                if not ((p % 2 == 1) or (p == P - 1)):
                    return
                nrows = 104 if p % 2 == 1 else D_OUT
                o = sb.tile([104, FDP], F16, tag="o")
                if p in OUT_DRAIN_ON_ACT:
                    nc.scalar.activation(o[:nrows, :fd], p2[:nrows, :fd],
                                         Act.Identity)
                else:
                    nc.vector.tensor_copy(o[:nrows, :fd], p2[:nrows, :fd])
                kp = p // 2
                ow = fd if p % 2 == 1 else _pairs[p]
                eng = nc.gpsimd if kp % 2 == 0 else nc.sync
                eng.dma_start(yT[:, kp * FDP:kp * FDP + ow], o[:, :ow])

            # Pre-bank work INSIDE the junk warmup window: batch 0/1 x
            # data lands at ~7-9us while the PE clock ramp runs, so
            # ACT/DVE chew through drain work during the otherwise-dead
            # head: stage-0 of pairs 0-2 (bounded by sb bufs=3), stage-1
            # of pairs 0-1, and the ENTIRE remainder pair REM (its own
            # x DMA + dedicated tile tags) so the pipeline tail ends on
            # a full pair and the last out-DMA isn't serialized behind
            # the remainder chain.
            PRE = min(3, P - 1)
            PRE1 = min(2, P - 1)
            stage_load(0)
            batch_tiles["w0"] = st[0]["xt"][:, 0:D_HID]
            st[REM] = {"xt": xtr, "base": _pstarts[REM]}
            for g in range(PRE):
                if g > 0:
                    stage_load(g)
                stage0_mm(g)
                stage0_elu(g)
                junk_mms(N_WARMUP_MM // PRE)
            # wb (w1/w2) issues third on sync, behind batch 0 and 1 —
            # only needed by the first L1 matmul (~13us)
            nc.sync.dma_start(wb_sb[:], wb[:])
            stage0_mm(REM)
            stage0_elu(REM)
            for g in range(PRE1):
                stage1_mm(g)
                stage1_elu(g)
            stage1_mm(REM)
            stage1_elu(REM)
            stage2(REM)

            for pp in range(2, (P - 1) + 3):
                if PRE <= pp < P - 1:
                    stage_load(pp)
                if PRE <= pp - 1 < P - 1:
                    stage0_mm(pp - 1)
                    stage0_elu(pp - 1)
                if PRE1 <= pp - 2 < P - 1:
                    stage1_mm(pp - 2)
                    stage1_elu(pp - 2)
                if 0 <= pp - 3 < P - 1:
                    stage2(pp - 3)
                if pp == P:
                    # hold the PE clock through the pipeline-drain tail:
                    # fill its drain-wait gap with junk into the now-idle
                    # ps0 pool so the final L2 matmuls run at 2.4 GHz.
                    ptail = ps0.tile([D_HID, FDP], F32, tag="p0")
                    for _ in range(3):
                        nc.tensor.matmul(ptail[:D_OUT, :],
                                         junk_w[:], junk_x[:],
                                         start=True, stop=True)

    nc.compile()
    return nc


_prog_cache = []
last_result = None


def kernel(**inputs) -> np.ndarray:
    global last_result
    x = np.asarray(inputs["x"], np.float32)           # [50000, 128]
    W0 = np.asarray(inputs["W0"], np.float32).reshape(D_HID, D_IN)
    W1 = np.asarray(inputs["W1"], np.float32).reshape(D_HID, D_HID)
    W2 = np.asarray(inputs["W2"], np.float32).reshape(D_OUT, D_HID)

    n = x.shape[0]
    assert n == N_CORES * N_PER, f"unexpected node count {n}"

    import ml_dtypes
    xT16 = x.T.astype(np.float16)                            # [128, 50000]
    w0t = W0.T.astype(np.float16)                            # [128, 96]
    w1tb = W1.T.astype(ml_dtypes.bfloat16)                   # [96, 96]
    w2tb = W2.T.astype(ml_dtypes.bfloat16)                   # [96, 40]
    wb = np.ascontiguousarray(
        np.concatenate([w1tb, w2tb], axis=1))                # [96, 136]

    if not _prog_cache:
        _prog_cache.append(_build_program())
    nc = _prog_cache[0]

    in_maps = []
    for i in range(N_CORES):
        xwi = np.ascontiguousarray(
            np.concatenate([w0t, xT16[:, i * N_PER:(i + 1) * N_PER]], axis=1))
        in_maps.append(dict(xw=xwi, wb=wb))
    res = run_bass_kernel_spmd(nc, in_maps, list(range(N_CORES)))
    last_result = res
    out = np.empty((n, D_OUT), np.float32)
    for i in range(N_CORES):
        yt = np.asarray(res.results[i]["yT"], np.float32)  # [104, 3178]
        base = i * N_PER
        for kp in range((P + 1) // 2):
            c0 = kp * FDP
            g0 = 2 * kp
            w0_ = _pairs[g0]
            out[base + _pstarts[g0]:base + _pstarts[g0] + w0_] = \
                yt[0:D_OUT, c0:c0 + w0_].T
            if g0 + 1 < P:
                w1_ = _pairs[g0 + 1]
                out[base + _pstarts[g0 + 1]:base + _pstarts[g0 + 1] + w1_] = \
                    yt[64:64 + D_OUT, c0:c0 + w1_].T
    return out


if __name__ == "__main__":
    data = np.load("/tmp/gat_inputs.npz")
    y = kernel(**{k: data[k] for k in data.files})
    print("out", y.shape, y.dtype, "absmax", np.abs(y).max())


# revision 31
# speedup vs baseline: 1.0681x; 1.0681x over previous
"""Trainium2 Bass kernel for nn_GAT_87617332838818.  (original baseline)"""

import os
import sys

import numpy as np

for _p in ("/root/.axon_site/_ro/trn_rl_repo", "/opt/trn_rl_repo"):
    if os.path.isdir(_p) and _p not in sys.path:
        sys.path.append(_p)

import concourse.bass as bass
import concourse.tile as tile
from concourse import bacc, mybir
from concourse.bass_utils import run_bass_kernel_spmd

N_CORES = 8
N_PER = 6250            # 50000 / 8
D_IN = 128
D_HID = 96
D_OUT = 40
MM_N = 512              # matmul moving free-dim (1 PSUM bank)
FDP = 512               # group free-dim (1 PSUM bank)

F16 = mybir.dt.float16
BF16 = mybir.dt.bfloat16
F32 = mybir.dt.float32

Act = mybir.ActivationFunctionType
Alu = mybir.AluOpType

_pairs = [FDP] * (N_PER // FDP)
if N_PER % FDP:
    _pairs.append(N_PER % FDP)
P = len(_pairs)
_pstarts = [sum(_pairs[:i]) for i in range(P)]

R_DRAIN_ON_ACT = tuple((p, 0) for p in range(P) if p % 4 not in (2, 3))
OUT_DRAIN_ON_ACT = (11, 12)
# pairs 0 | 1-4 | 5-8 | 9-11; the remainder pair P-1 gets its own small
# head DMA and is processed entirely inside the warmup window.
X_BATCHES = [1, 4, 4, 3]
N_WARMUP_MM = int(os.environ.get("GAT_WARMUP", "12"))
REM = P - 1

_batch_of = {}
_b0 = 0
for _bi, _bn in enumerate(X_BATCHES):
    for _g in range(_b0, min(_b0 + _bn, P - 1)):
        _batch_of[_g] = _bi
    _b0 += _bn
assert _b0 >= P - 1


def _mm_splits(fd):
    """Split a pair-tick's fd into <=512 matmul chunks."""
    out = []
    j = 0
    while j < fd:
        out.append((j, min(j + MM_N, fd)))
        j += MM_N
    return out


def _build_program() -> bass.Bass:
    nc = bacc.Bacc(None, target_bir_lowering=False, debug=False)

    xw = nc.declare_dram_parameter("xw", [D_IN, D_HID + N_PER], F16,
                                   isOutput=False)
    wb = nc.declare_dram_parameter("wb", [D_HID, D_HID + D_OUT], BF16,
                                   isOutput=False)
    yT = nc.declare_dram_parameter("yT", [104, 3178], F16, isOutput=True)

    st = {}
    st_batch = {}
    batch_tiles = {}

    with tile.TileContext(nc) as tc:
        with (
            tc.tile_pool(name="consts", bufs=1) as consts,
            tc.tile_pool(name="x0", bufs=1) as x0pool,
            tc.tile_pool(name="xin", bufs=2) as xpool,
            tc.tile_pool(name="sb", bufs=3) as sb,
            tc.tile_pool(name="ps0", bufs=3, space="PSUM") as ps0,
            tc.tile_pool(name="ps1", bufs=3, space="PSUM") as ps1,
            tc.tile_pool(name="ps2", bufs=2, space="PSUM") as ps2,
        ):
            # --- PE warm-up on garbage SBUF during the DMA-bound head.
            junk_w = consts.tile([D_IN, D_OUT], F16, tag="junkw")
            junk_x = consts.tile([D_IN, MM_N], F16, tag="junkx")
            nc.gpsimd.memset(junk_w[:], 0.0)
            nc.gpsimd.memset(junk_x[:], 0.0)
            # remainder pair's x rides the gpsimd queue (cheap DMA issue)
            # in parallel with batch-0 on sync, just after the memsets so
            # the PE warmup isn't delayed behind the DMA issue
            xtr = x0pool.tile([D_IN, _pairs[P - 1]], F16, tag="xtr")
            nc.gpsimd.dma_start(xtr[:, :_pairs[P - 1]],
                                xw[:, D_HID + _pstarts[P - 1]:
                                   D_HID + _pstarts[P - 1] + _pairs[P - 1]])
            warm = ps2.tile([104, MM_N], F32, tag="p2")

            def junk_mms(n, moving=None):
                # moving=real-x makes the junk DEPEND on the batch-0 DMA,
                # so the scheduler cannot hoist junk ahead of the first
                # real L0 matmul (which would delay ACT's first drain).
                mv = junk_x[:] if moving is None else moving
                for _ in range(n):
                    nc.tensor.matmul(warm[:D_OUT], junk_w[:], mv,
                                     start=True, stop=True)

            wb_sb = consts.tile([D_HID, D_HID + D_OUT], BF16, tag="wb")
            w1_sb = wb_sb[:, :D_HID]
            w2_sb = wb_sb[:, D_HID:D_HID + D_OUT]

            def relu_drain(out_ap, psum_ap, on_act):
                """out = max(psum, 0), PSUM -> SBUF bf16."""
                if on_act:
                    nc.scalar.activation(out_ap, psum_ap, Act.Relu)
                else:
                    nc.vector.tensor_scalar_max(out_ap, psum_ap, 0.0)

            def exp_elu(p, lyr, psum, fd):
                # the remainder pair gets dedicated tags so its head-time
                # drains never WAR-wait on the main pipeline's slot reuse
                sfx = "z" if p == REM else ""
                e = sb.tile([D_HID, fd], BF16, tag=f"e{lyr}{sfx}")
                r = sb.tile([D_HID, fd], BF16, tag=f"r{lyr}{sfx}")
                for j0, j1 in _mm_splits(fd):
                    nc.scalar.activation(e[:, j0:j1], psum[:, j0:j1], Act.Exp)
                    relu_drain(r[:, j0:j1], psum[:, j0:j1],
                               (p, lyr) in R_DRAIN_ON_ACT)
                t = sb.tile([D_HID, fd], BF16, tag=f"t{lyr}{sfx}")
                nc.vector.tensor_scalar(t[:, :fd], e[:, :fd], 1.0, -1.0,
                                        Alu.min, Alu.add)
                return r, t

            def stage_load(p):
                bi = _batch_of[p]
                if p > 0 and _batch_of[p - 1] == bi:
                    st[p] = st_batch[bi]
                    return
                p1_ = p
                while p1_ + 1 < P - 1 and _batch_of[p1_ + 1] == bi:
                    p1_ += 1
                lo = _pstarts[p] + (0 if bi else -D_HID)   # batch 0 incl. w0
                hi = _pstarts[p1_] + _pairs[p1_]
                cols = hi - lo
                pool = x0pool if bi == 0 else xpool
                width = D_HID + FDP * X_BATCHES[0] if bi == 0 else FDP * 4
                xt = pool.tile([D_IN, width], F16,
                               tag=("xt0" if bi == 0 else "xt"))
                nc.sync.dma_start(xt[:, :cols], xw[:, D_HID + lo:D_HID + hi])
                st_batch[bi] = {"xt": xt, "base": lo}
                st[p] = st_batch[bi]

            def stage0_mm(p):
                fd = _pairs[p]
                s = dict(st[p])
                st[p] = s
                xo = _pstarts[p] - s["base"]
                w0_sb = batch_tiles["w0"]
                p0 = ps0.tile([D_HID, FDP], F32, tag="p0")
                for j0, j1 in _mm_splits(fd):
                    nc.tensor.matmul(p0[:, j0:j1], w0_sb,
                                     s["xt"][:, xo + j0:xo + j1],
                                     start=True, stop=True)
                s["p0"] = p0

            def stage0_elu(p):
                s = st[p]
                s["r1"], s["t1"] = exp_elu(p, 0, s.pop("p0"), _pairs[p])

            def stage1_mm(p):
                fd = _pairs[p]
                s = st[p]
                p1 = ps1.tile([D_HID, FDP], F32, tag="p1")
                for j0, j1 in _mm_splits(fd):
                    nc.tensor.matmul(p1[:, j0:j1], w1_sb, s["r1"][:, j0:j1],
                                     start=True, stop=False)
                    nc.tensor.matmul(p1[:, j0:j1], w1_sb, s["t1"][:, j0:j1],
                                     start=False, stop=True)
                s["p1"] = p1

            def stage1_elu(p):
                s = st[p]
                s["r2"], s["t2"] = exp_elu(p, 1, s.pop("p1"), _pairs[p])

            pair_state = {}

            def stage2(p):
                fd = _pairs[p]
                s = st.pop(p)
                if p % 2 == 0:
                    p2 = ps2.tile([104, FDP], F32, tag="p2")
                    pair_state[p // 2] = p2
                    rows = slice(0, D_OUT)
                else:
                    p2 = pair_state[p // 2]
                    rows = slice(64, 64 + D_OUT)
                nc.tensor.matmul(p2[rows, :fd], w2_sb, s["r2"][:, :fd],
                                 start=True, stop=False)
                nc.tensor.matmul(p2[rows, :fd], w2_sb, s["t2"][:, :fd],
                                 start=False, stop=True)
                if not ((p % 2 == 1) or (p == P - 1)):
                    return
                nrows = 104 if p % 2 == 1 else D_OUT
                o = sb.tile([104, FDP], F16, tag="o")
                if p in OUT_DRAIN_ON_ACT:
                    nc.scalar.activation(o[:nrows, :fd], p2[:nrows, :fd],
                                         Act.Identity)
                else:
                    nc.vector.tensor_copy(o[:nrows, :fd], p2[:nrows, :fd])
                kp = p // 2
                ow = fd if p % 2 == 1 else _pairs[p]
                eng = nc.gpsimd if kp % 2 == 0 else nc.sync
                eng.dma_start(yT[:, kp * FDP:kp * FDP + ow], o[:, :ow])

            # Pre-bank work INSIDE the junk warmup window: batch 0/1 x
            # data lands at ~7-9us while the PE clock ramp runs, so
            # ACT/DVE chew through drain work during the otherwise-dead
            # head: stage-0 of pairs 0-2 (bounded by sb bufs=3), stage-1
            # of pairs 0-1, and the ENTIRE remainder pair REM (its own
            # x DMA + dedicated tile tags) so the pipeline tail ends on
            # a full pair and the last out-DMA isn't serialized behind
            # the remainder chain.
            PRE = min(3, P - 1)
            PRE1 = min(2, P - 1)
            stage_load(0)
            batch_tiles["w0"] = st[0]["xt"][:, 0:D_HID]
            st[REM] = {"xt": xtr, "base": _pstarts[REM]}
            for g in range(PRE):
                if g > 0:
                    stage_load(g)
                stage0_mm(g)
                stage0_elu(g)
                junk_mms(N_WARMUP_MM // PRE)
            # wb (w1/w2) issues third on sync, behind batch 0 and 1 —
            # only needed by the first L1 matmul (~13us)
            nc.sync.dma_start(wb_sb[:], wb[:])
            stage0_mm(REM)
            stage0_elu(REM)
            for g in range(PRE1):
                stage1_mm(g)
                stage1_elu(g)
            stage1_mm(REM)
            stage1_elu(REM)
            stage2(REM)

            for pp in range(2, (P - 1) + 3):
                if PRE <= pp < P - 1:
                    stage_load(pp)
                if PRE <= pp - 1 < P - 1:
                    stage0_mm(pp - 1)
                    stage0_elu(pp - 1)
                if PRE1 <= pp - 2 < P - 1:
                    stage1_mm(pp - 2)
                    stage1_elu(pp - 2)
                if 0 <= pp - 3 < P - 1:
                    stage2(pp - 3)
                if pp == P:
                    # hold the PE clock through the pipeline-drain tail:
                    # fill its drain-wait gap with junk into the now-idle
                    # ps0 pool so the final L2 matmuls run at 2.4 GHz.
                    ptail = ps0.tile([D_HID, FDP], F32, tag="p0")
                    for _ in range(3):
                        nc.tensor.matmul(ptail[:D_OUT, :],
                                         junk_w[:], junk_x[:],
                                         start=True, stop=True)

    nc.compile()
    return nc


_prog_cache = []
last_result = None


def kernel(**inputs) -> np.ndarray:
    global last_result
    x = np.asarray(inputs["x"], np.float32)           # [50000, 128]
    W0 = np.asarray(inputs["W0"], np.float32).reshape(D_HID, D_IN)
    W1 = np.asarray(inputs["W1"], np.float32).reshape(D_HID, D_HID)
    W2 = np.asarray(inputs["W2"], np.float32).reshape(D_OUT, D_HID)

    n = x.shape[0]
    assert n == N_CORES * N_PER, f"unexpected node count {n}"

    import ml_dtypes
    xT16 = x.T.astype(np.float16)                            # [128, 50000]
    w0t = W0.T.astype(np.float16)                            # [128, 96]
    w1tb = W1.T.astype(ml_dtypes.bfloat16)                   # [96, 96]
    w2tb = W2.T.astype(ml_dtypes.bfloat16)                   # [96, 40]
    wb = np.ascontiguousarray(
        np.concatenate([w1tb, w2tb], axis=1))                # [96, 136]

    if not _prog_cache:
        _prog_cache.append(_build_program())
    nc = _prog_cache[0]

    in_maps = []
    for i in range(N_CORES):
        xwi = np.ascontiguousarray(
            np.concatenate([w0t, xT16[:, i * N_PER:(i + 1) * N_PER]], axis=1))
        in_maps.append(dict(xw=xwi, wb=wb))
    res = run_bass_kernel_spmd(nc, in_maps, list(range(N_CORES)))
    last_result = res
    out = np.empty((n, D_OUT), np.float32)
    for i in range(N_CORES):
        yt = np.asarray(res.results[i]["yT"], np.float32)  # [104, 3178]
        base = i * N_PER
        for kp in range((P + 1) // 2):
            c0 = kp * FDP
            g0 = 2 * kp
            w0_ = _pairs[g0]
            out[base + _pstarts[g0]:base + _pstarts[g0] + w0_] = \
                yt[0:D_OUT, c0:c0 + w0_].T
            if g0 + 1 < P:
                w1_ = _pairs[g0 + 1]
                out[base + _pstarts[g0 + 1]:base + _pstarts[g0 + 1] + w1_] = \
                    yt[64:64 + D_OUT, c0:c0 + w1_].T
    return out


if __name__ == "__main__":
    data = np.load("/tmp/gat_inputs.npz")
    y = kernel(**{k: data[k] for k in data.files})
    print("out", y.shape, y.dtype, "absmax", np.abs(y).max())
